# revision 2
# baseline (speedup 1.0000x reference)
"""Trainium2 Bass kernel for nn_BigNet: 1000x (Linear(100,100)+ReLU) -> Linear(100,10).

fp16 DVE-only design (data-parallel, batch 8192 -> 1024 cols/core):
  - All matmul inputs fp16 (end-to-end max rel err ~4.4e-3 vs 2e-2 gate).
    PSUM accumulates fp32.
  - 2 chunks x 512 cols per layer: chunk 0 evicted by ScalarE activation
    (Relu), chunk 1 by VectorE tensor_scalar_max, running in parallel.
    512/512 balances ACT (FD+315cyc)/1.2GHz vs DVE (FD+152cyc)/0.96GHz.
    (GPSIMD cannot access PSUM; measured layer-sim: ~683ns/layer.)
  - Weights stream from HBM as fp16 groups of 8 layers on the SP queue,
    double-buffered 4 deep.
"""

import sys

if "/opt/trn_rl_repo" not in sys.path:
    sys.path.insert(0, "/opt/trn_rl_repo")

import numpy as np

N_LAYERS, D, D_OUT, B, N_CORES = 1000, 100, 10, 8192, 8
K = D + 1  # augmented contraction dim (ones row carries the bias)
B_CORE = B // N_CORES
CW = 512  # chunk width; PSUM bank limit = 512 fp32 cols
N_CHUNKS = B_CORE // CW
W_PER_DMA = 8

_BUILT = {}


def _build():
    import concourse.bacc as bacc
    import concourse.mybir as mybir
    from concourse.tile import TileContext

    f16 = mybir.dt.float16
    f32 = mybir.dt.float32

    nc = bacc.Bacc(None, target_bir_lowering=False)
    n_groups = N_LAYERS // W_PER_DMA
    wt_e = nc.dram_tensor(
        "wt", [n_groups, K, D * W_PER_DMA], f16, kind="ExternalInput"
    )
    xt_e = nc.dram_tensor("xt", [K, B_CORE], f16, kind="ExternalInput")
    wft_e = nc.dram_tensor("wft", [K, D_OUT], f16, kind="ExternalInput")
    out_e = nc.dram_tensor("out", [D_OUT, B_CORE], f32, kind="ExternalOutput")

    with TileContext(nc) as tc:
        with (
            tc.tile_pool(name="h", bufs=1) as hpool,
            tc.tile_pool(name="w", bufs=4) as wpool,
            tc.tile_pool(name="ps", bufs=1, space="PSUM") as pspool,
            tc.tile_pool(name="misc", bufs=1) as mpool,
        ):
            # Prefetch the first two weight groups before activations.
            wtiles0 = []
            for g0 in range(2):
                wt_t = wpool.tile([K, D * W_PER_DMA], f16, tag="w", name="wtile")
                nc.sync.dma_start(wt_t[:], wt_e[g0])
                wtiles0.append(wt_t)

            wf_tile = mpool.tile([K, D_OUT], f16, tag="wf")
            nc.sync.dma_start(wf_tile[:], wft_e[:])

            # Ping-pong activation tiles [K, 1024] fp16 (2KB/partition, so
            # 512-col halves sit at 4B-aligned offsets -> DVE 2x mode).
            ha = hpool.tile([K, B_CORE], f16, tag="h0", name="h0")
            hb = hpool.tile([K, B_CORE], f16, tag="h1", name="h1")
            nc.sync.dma_start(ha[:], xt_e[:])
            nc.sync.dma_start(hb[D:K, :], xt_e[D:K, :])  # ones row

            cur, nxt = ha, hb
            for l in range(N_LAYERS):
                j = l % W_PER_DMA
                if j == 0:
                    g = l // W_PER_DMA
                    if g < 2:
                        wtile = wtiles0[g]
                    else:
                        wtile = wpool.tile(
                            [K, D * W_PER_DMA], f16, tag="w", name="wtile"
                        )
                        nc.sync.dma_start(wtile[:], wt_e[g])
                wsl = wtile[:, j * D : (j + 1) * D]
                for c in range(N_CHUNKS):
                    ps = pspool.tile([D, CW], f32, tag=f"ps{c}", name=f"ps{c}")
                    nc.tensor.matmul(
                        ps[:], wsl, cur[:, c * CW : (c + 1) * CW],
                        start=True, stop=True,
                    )
                    if c == 0:
                        nc.scalar.activation(
                            nxt[0:D, c * CW : (c + 1) * CW],
                            ps[:],
                            mybir.ActivationFunctionType.Relu,
                        )
                    else:
                        nc.vector.tensor_scalar_max(
                            nxt[0:D, c * CW : (c + 1) * CW], ps[:], 0.0
                        )
                cur, nxt = nxt, cur

            # Final Linear(100 -> 10), no ReLU; ACT engine (idle) evicts.
            out_sb = mpool.tile([D_OUT, B_CORE], f32, tag="out")
            for c in range(N_CHUNKS):
                ps = pspool.tile([D_OUT, CW], f32, tag=f"ps{c}", name=f"psf{c}")
                nc.tensor.matmul(
                    ps[:], wf_tile[:], cur[:, c * CW : (c + 1) * CW],
                    start=True, stop=True,
                )
                nc.scalar.copy(out_sb[:, c * CW : (c + 1) * CW], ps[:])
            nc.sync.dma_start(out_e[:], out_sb[:])

    nc.finalize()
    return nc


def _get_nc():
    nc = _BUILT.get("v2")
    if nc is None:
        nc = _build()
        _BUILT["v2"] = nc
    return nc


def _prep_inputs(x, W, b, Wf, bf):
    x = np.asarray(x, dtype=np.float32)
    W = np.asarray(W, dtype=np.float32)
    b = np.asarray(b, dtype=np.float32)
    Wf = np.asarray(Wf, dtype=np.float32)
    bf = np.asarray(bf, dtype=np.float32)

    # wt[g, p, j*D + m] = Waug[g*W_PER_DMA + j, p, m], Waug[l] = [W[l].T ; b[l]]
    waug = np.concatenate([W.transpose(0, 2, 1), b[:, None, :]], axis=1)
    n_groups = N_LAYERS // W_PER_DMA
    wt = np.ascontiguousarray(
        waug.reshape(n_groups, W_PER_DMA, K, D)
        .transpose(0, 2, 1, 3)
        .reshape(n_groups, K, W_PER_DMA * D)
    ).astype(np.float16)

    xt = np.empty((K, B), dtype=np.float16)
    xt[:D] = x.T.astype(np.float16)
    xt[D] = 1.0
    xt_cores = [
        np.ascontiguousarray(xt[:, i * B_CORE : (i + 1) * B_CORE])
        for i in range(N_CORES)
    ]

    wft = np.ascontiguousarray(
        np.concatenate([Wf.T, bf[None, :]], axis=0)
    ).astype(np.float16)
    return wt, xt_cores, wft


def run(x, W, b, Wf, bf, mm_dtype=None, trace=False):
    from concourse.bass_utils import run_bass_kernel_spmd

    nc = _get_nc()
    wt, xt_cores, wft = _prep_inputs(x, W, b, Wf, bf)
    in_maps = [
        {"wt": wt, "xt": xt_cores[i], "wft": wft} for i in range(N_CORES)
    ]
    res = run_bass_kernel_spmd(
        nc, in_maps, core_ids=list(range(N_CORES)), trace=trace
    )
    out = np.concatenate([res.results[i]["out"] for i in range(N_CORES)], axis=1)
    return np.ascontiguousarray(out.T, dtype=np.float32), res


def kernel(x, W, b, Wf, bf):
    out, _ = run(x, W, b, Wf, bf)
    return out
